# revision 53
# baseline (speedup 1.0000x reference)
"""Trainium2 Bass kernel for causal self-attention with RoPE.

Shapes: x (2, 2048, 2048), 16 heads x 128 head_dim.
Sharding: 8 cores = 2 batch x 4 head-groups (4 heads per core).
Each core computes q/k/v projections for its heads, RoPE, causal-masked
softmax attention, and a partial output projection (its head columns of
wo); the host sums the 4 partials per batch element.

Layout strategy (per core):
  - all matmul operands in bf16; accumulation fp32 in PSUM.
  - q,k built in transposed layout (head_dim on partitions, t free) so
    RoPE and the score matmuls need no on-device transposes.
  - the v projection is interleaved into the q/k pass per 512-query
    slice (x^T is streamed from HBM exactly once).
  - scores computed as s^T (keys x q) per 256-query group.  The causal
    structure is exploited at query-HALF granularity: for the diagonal
    key chunk whose lower query half is fully masked, only a 128-wide
    matmul is issued ("narrow" block).
  - softmax denominators: instead of one ones-matmul per key block
    (which costs as much PE streaming as p@v itself), pt blocks are
    pre-folded in quads on the DVE and a single 256-wide ones-matmul
    per quad accumulates l in PSUM.  o and l live in one PSUM bank as
    a single interleaved accumulation group (pv opens it; all later
    pv/l matmuls accumulate element-wise).
  - p@v and l matmuls trail the score/exp stream by up to three quads
    so group boundaries have deferred PE work to hide the exp latency;
    big query groups bracket the stream (tiny j=0/1 mid-stream, j=7
    last so the final drain has padding work).
  - output projection per query group is interleaved right after its
    last head finalizes; PSUM evacuations alternate ScalarE/VectorE
    and the output is written in bf16 (host sums partials in fp32).
  - startup: dummy matmuls on a memset tile warm the PE clock (HAM)
    and bridge the DMA wait; wq/xt arrive in need-order pieces, each
    critical piece ordered so no big-descriptor transfer competes
    with it (DMA queues are FIFO; bus share scales with descriptor
    size = piece bytes per partition).
"""

import sys
from contextlib import ExitStack

if "/opt/trn_rl_repo" not in sys.path:
    sys.path.insert(0, "/opt/trn_rl_repo")

import numpy as np
import ml_dtypes

import concourse.bacc as bacc
import concourse.mybir as mybir
import concourse.tile as tile
from concourse.bass_utils import run_bass_kernel_spmd

B, T, D, NH, HD = 2, 2048, 2048, 16, 128
HPC = 4              # heads per core
PAIR = 256           # queries per attention group
NPAIR = T // PAIR    # 8
NCHUNK = T // HD     # 16 key chunks of 128
PROJ = 512           # queries per projection slice
NSLICE = T // PROJ   # 4
BF16 = mybir.dt.bfloat16
F32 = mybir.dt.float32
NPBF16 = np.dtype(ml_dtypes.bfloat16)


def _mask_structure(mask):
    """Classify (query-group, key-chunk) blocks at query-half granularity.

    Returns (statuses, maskt):
      statuses[j] = list of blocks, full-width blocks first (ascending
        chunk), then narrow blocks.  Each block is a dict:
          {"c": chunk, "kind": "full"|"narrow", "lo": mi, "hi": mi}
        where mi is a mask-tile index or -1 (half fully visible).
        "narrow" means the lower query half is fully masked; only the
        upper 128 queries are computed.
      maskt: packed (128, nmask, 128) array of deduplicated transposed
        exp-mask tiles ([keys, queries]).
    """
    statuses = []
    tiles = {}
    tile_list = []

    def tile_idx(sub):
        key = sub.tobytes()
        mi = tiles.get(key)
        if mi is None:
            assert np.all(sub <= 64.0), "additive mask too large for exp-mask"
            mi = len(tile_list)
            tiles[key] = mi
            tile_list.append(np.exp(np.ascontiguousarray(sub.T)))
        return mi

    def half_state(h):
        if np.all(h <= -1e8):
            return "skip"
        if np.all(h == 0.0):
            return "free"
        return "part"

    for j in range(NPAIR):
        full, narrow = [], []
        for c in range(NCHUNK):
            ks = slice(c * HD, (c + 1) * HD)
            lo = mask[j * PAIR:j * PAIR + HD, ks]
            hi = mask[j * PAIR + HD:(j + 1) * PAIR, ks]
            slo, shi = half_state(lo), half_state(hi)
            if slo == "skip" and shi == "skip":
                continue
            if slo == "skip":
                narrow.append({"c": c, "kind": "narrow", "lo": -1,
                               "hi": -1 if shi == "free" else tile_idx(hi)})
            else:
                full.append({"c": c, "kind": "full",
                             "lo": -1 if slo == "free" else tile_idx(lo),
                             "hi": -1 if shi == "free" else tile_idx(hi)})
        if not full:
            # pv group opener must cover all 256 o columns
            b = narrow.pop(0)
            c = b["c"]
            full.append({"c": c, "kind": "full",
                         "lo": tile_idx(mask[j * PAIR:j * PAIR + HD,
                                             c * HD:(c + 1) * HD]),
                         "hi": b["hi"]})
        assert full or narrow, f"query group {j} fully masked"
        statuses.append(full + narrow)
    nmask = max(1, len(tile_list))
    maskt = np.zeros((HD, nmask, HD), np.float32)
    for i, t in enumerate(tile_list):
        maskt[:, i, :] = t
    return statuses, maskt


def _build_program(statuses, nmask):
    nc = bacc.Bacc(None, target_bir_lowering=False)

    # All inputs are laid out on the host so that each SBUF partition's
    # content is one contiguous DRAM run (large descriptors keep the
    # DMA queues at full bus rate).
    xt_d = nc.dram_tensor("xts", [NSLICE, HD, NCHUNK, PROJ], BF16,
                          kind="ExternalInput")
    wq_d = nc.dram_tensor("wqt", [HD, NCHUNK, HPC * HD], BF16, kind="ExternalInput")
    wk_d = nc.dram_tensor("wkt", [HD, NCHUNK, HPC * HD], BF16, kind="ExternalInput")
    wv_d = nc.dram_tensor("wvt", [HD, NCHUNK, HPC * HD], BF16, kind="ExternalInput")
    wo_d = nc.dram_tensor("wot", [HD, HPC, D], BF16, kind="ExternalInput")
    cs_d = nc.dram_tensor("cs", [NSLICE, HD, 2, PROJ], BF16, kind="ExternalInput")
    mk_d = nc.dram_tensor("maskt", [HD, nmask, HD], BF16, kind="ExternalInput")
    ones_d = nc.dram_tensor("ones_sq", [HD, HD], BF16, kind="ExternalInput")
    out_d = nc.dram_tensor("out", [T, D], BF16, kind="ExternalOutput")

    xt_ap = xt_d.ap()
    wq_ap = wq_d.ap()
    wk_ap = wk_d.ap()
    wv_ap = wv_d.ap()
    wo_ap = wo_d.ap()
    EXP = mybir.ActivationFunctionType.Exp

    with tile.TileContext(nc) as tc, ExitStack() as top:
        # ---- PE warmup: dummy matmuls while the first DMAs land ----
        # The HAM clock gate holds the PE at 1.2 GHz until it has seen
        # ~3.4us of sustained activity; these no-dependency matmuls
        # burn that window during the DMA wait so the first real
        # matmuls run at 2.4 GHz.
        constp = top.enter_context(tc.tile_pool(name="const", bufs=1))
        ones_sb = constp.tile([HD, HD], BF16)
        wsrc = constp.tile([HD, HD], BF16)
        nc.gpsimd.memset(wsrc[:], 0.0)
        with ExitStack() as wh:
            wpp = wh.enter_context(tc.tile_pool(name="wpp", bufs=1, space="PSUM"))
            wps = wpp.tile([HD, HD], F32)
            for _ in range(52):
                nc.tensor.matmul(wps[:], wsrc[:], wsrc[:], start=True, stop=True)

        qkp = top.enter_context(tc.tile_pool(name="qkp", bufs=1))
        # q heads at [:, h, :], k heads at [:, 4+h, :]
        qk_sb = qkp.tile([HD, 2 * HPC, T], BF16)
        vap = top.enter_context(tc.tile_pool(name="vap", bufs=1))
        v_all = vap.tile([HD, NCHUNK, HPC * HD], BF16)
        ctxp = top.enter_context(tc.tile_pool(name="ctxp", bufs=1))
        ctx_sb = ctxp.tile([HD, HPC, T], BF16)
        wop = top.enter_context(tc.tile_pool(name="wop", bufs=1))
        wo_sb = wop.tile([HD, HPC, D], BF16)
        mkpre = top.enter_context(tc.tile_pool(name="mkpre", bufs=1))
        mk_sb = mkpre.tile([HD, nmask, HD], BF16)

        # ---- combined q/k/v projection pass (+ fused RoPE) ----
        with ExitStack() as ph:
            wp = ph.enter_context(tc.tile_pool(name="wp", side="right", bufs=1))
            wvp = ph.enter_context(tc.tile_pool(name="wvp", side="right", bufs=1))
            xtp = ph.enter_context(tc.tile_pool(name="xtp", side="right", bufs=2))
            csp = ph.enter_context(tc.tile_pool(name="csp", side="right", bufs=2))
            ropep = ph.enter_context(tc.tile_pool(name="ropep", side="right", bufs=2))
            pps = ph.enter_context(tc.tile_pool(name="pps", bufs=6, space="PSUM"))
            vps = ph.enter_context(tc.tile_pool(name="vps", bufs=2, space="PSUM"))
            wqk_sb = wp.tile([HD, 2, NCHUNK, HPC * HD], BF16)
            wv_sb = wvp.tile([HD, NCHUNK, HPC * HD], BF16)

            # Startup choreography: the bus is saturated for the first
            # ~40us; per-queue rate scales with descriptor size
            # (bytes-per-partition of the piece), and a queue round-robins
            # batches across ALL its queued transfers, so a critical
            # piece finishes promptly only if nothing is queued behind it.
            # Each need-ordered piece therefore rides its own queue
            # (scalar/gpsimd for weights, sync/vector for activations).
            nc.scalar.dma_start(wqk_sb[:, 0, 0:2, :], wq_ap[:, 0:2, :])
            nc.scalar.dma_start(wqk_sb[:, 0, 2:8, :], wq_ap[:, 2:8, :])
            nc.scalar.dma_start(wqk_sb[:, 0, 8:, :], wq_ap[:, 8:, :])
            cs0 = csp.tile([HD, 2, PROJ], BF16, tag="cs")
            nc.gpsimd.dma_start(cs0[:], cs_d.ap()[0])
            nc.scalar.dma_start(wv_sb[:], wv_ap[:])

            for ns in range(NSLICE):
                tsl = slice(ns * PROJ, (ns + 1) * PROJ)
                xt = xtp.tile([HD, NCHUNK, PROJ], BF16, tag="xt")
                if ns == 0:
                    cs_sl = cs0
                    nc.sync.dma_start(xt[:, 0:2, :], xt_ap[ns, :, 0:2, :])
                    nc.sync.dma_start(xt[:, 2:8, :], xt_ap[ns, :, 2:8, :])
                    nc.sync.dma_start(xt[:, 8:, :], xt_ap[ns, :, 8:, :])
                    # wk follows the critical x pieces on sync (its 16KB
                    # descriptors would otherwise starve them), split so
                    # the k accumulation can start on the first half
                    nc.sync.dma_start(wqk_sb[:, 1, 0:8, :], wk_ap[:, 0:8, :])
                    nc.sync.dma_start(wqk_sb[:, 1, 8:, :], wk_ap[:, 8:, :])
                else:
                    cs_sl = csp.tile([HD, 2, PROJ], BF16, tag="cs")
                    nc.sync.dma_start(xt[:], xt_ap[ns])
                    nc.scalar.dma_start(cs_sl[:], cs_d.ap()[ns])
                if ns == 2:
                    # tiny attention constants go late: their 256B
                    # descriptors would eat DMA-engine slots during the
                    # startup-critical window
                    nc.gpsimd.dma_start(ones_sb[:], ones_d[:])
                    nc.gpsimd.dma_start(mk_sb[:], mk_d[:])
                if ns == 1:
                    # wo rides the sync queue once the startup-critical
                    # pieces are through (needed only by the first
                    # output projection, ~170us in)
                    nc.sync.dma_start(wo_sb[:], wo_ap[:])

                def rope(ps, wsel, h):
                    # RoPE: dst = raw*C + swap(raw)*S.  The swap is
                    # materialized by two ScalarE half-copies; VectorE
                    # does two multiplies and one add per tile.
                    dst = qk_sb[:, wsel * HPC + h, tsl]
                    sw = ropep.tile([HD, PROJ], F32, tag="sw")
                    nc.scalar.copy(sw[0:64, :], ps[64:128, :])
                    nc.scalar.copy(sw[64:128, :], ps[0:64, :])
                    tb = ropep.tile([HD, PROJ], F32, tag="tb")
                    nc.vector.tensor_mul(dst, ps[:], cs_sl[:, 0, :])
                    nc.vector.tensor_mul(tb[:], sw[:], cs_sl[:, 1, :])
                    nc.vector.tensor_add(dst, dst, tb[:])

                if ns == 0:
                    # startup: q accumulations run in chunk phases that
                    # match the wq/xt DMA pieces, all heads per phase
                    qtiles = [pps.tile([HD, PROJ], F32, tag="ps",
                                       name=f"qps{h}")
                              for h in range(HPC)]
                    wrm = pps.tile([HD, PROJ], F32, tag="ps", name="wrm")
                    for a, b in ((0, 2), (2, 8), (8, NCHUNK)):
                        for h in range(HPC):
                            hs = slice(h * HD, (h + 1) * HD)
                            for k in range(a, b):
                                nc.tensor.matmul(
                                    qtiles[h][:], wqk_sb[:, 0, k, hs],
                                    xt[:, k, :],
                                    start=(k == 0), stop=(k == NCHUNK - 1),
                                )
                            if b == NCHUNK:
                                rope(qtiles[h], 0, h)
                        if b == 2:
                            # keep the PE busy (and the HAM clock warm)
                            # while the next wq/xt pieces land
                            for _ in range(52):
                                nc.tensor.matmul(wrm[:, 0:HD], wsrc[:],
                                                 wsrc[:], start=True,
                                                 stop=True)
                    ktiles = [pps.tile([HD, PROJ], F32, tag="ps",
                                       name=f"kps{h}")
                              for h in range(HPC)]
                    for a, b in ((0, 8), (8, NCHUNK)):
                        for h in range(HPC):
                            hs = slice(h * HD, (h + 1) * HD)
                            for k in range(a, b):
                                nc.tensor.matmul(
                                    ktiles[h][:], wqk_sb[:, 1, k, hs],
                                    xt[:, k, :],
                                    start=(k == 0), stop=(k == NCHUNK - 1),
                                )
                            if b == NCHUNK:
                                rope(ktiles[h], 1, h)
                else:
                    for wsel in range(2):
                        for h in range(HPC):
                            ps = pps.tile([HD, PROJ], F32, tag="ps")
                            hs = slice(h * HD, (h + 1) * HD)
                            for k in range(NCHUNK):
                                nc.tensor.matmul(
                                    ps[:],
                                    wqk_sb[:, wsel, k, hs],
                                    xt[:, k, :],
                                    start=(k == 0),
                                    stop=(k == NCHUNK - 1),
                                )
                            rope(ps, wsel, h)

                # ---- v projection for this slice (x^T stationary) ----
                for tc2 in range(PROJ // HD):
                    vp_t = vps.tile([HD, HPC * HD], F32, tag="vps")
                    for k in range(NCHUNK):
                        nc.tensor.matmul(
                            vp_t[:],
                            xt[:, k, tc2 * HD:(tc2 + 1) * HD],
                            wv_sb[:, k, :],
                            start=(k == 0),
                            stop=(k == NCHUNK - 1),
                        )
                    nc.scalar.copy(v_all[:, ns * (PROJ // HD) + tc2, :], vp_t[:])

        # ---- attention + interleaved output projection ----
        with ExitStack() as ph:
            ptp = ph.enter_context(tc.tile_pool(name="ptp", side="right", bufs=2))
            lrp = ph.enter_context(tc.tile_pool(name="lrp", side="right", bufs=2))
            lqp = ph.enter_context(tc.tile_pool(name="lqp", side="right", bufs=4))
            evp = ph.enter_context(tc.tile_pool(name="evp", side="right", bufs=2))
            sps = ph.enter_context(tc.tile_pool(name="sps", bufs=2, space="PSUM"))
            olps = ph.enter_context(tc.tile_pool(name="olps", bufs=2, space="PSUM"))
            wops = ph.enter_context(tc.tile_pool(name="wops", bufs=2, space="PSUM"))

            def finalize(st):
                # DVE fast-recip of the (already partition-broadcast) row
                # sums, then one DVE multiply into ctx
                lr = lrp.tile([HD, PAIR], F32, tag="lr")
                nc.vector.reciprocal_approx_fast(lr[:], st["l"])
                nc.vector.tensor_mul(
                    ctx_sb[:, st["h"], st["qsl"]], st["o"], lr[:]
                )

            def emit_outproj(j):
                # output projection for query group j (all heads final);
                # PSUM evacuations alternate ScalarE/VectorE, output DMA
                # is one 512KB transfer per 128-query tile, queues
                # alternating sync/gpsimd.
                for tck in range(PAIR // HD):
                    tsl = slice(j * PAIR + tck * HD, j * PAIR + (tck + 1) * HD)
                    ev = evp.tile([HD, D], BF16, tag="ev")
                    for es in range(D // 512):
                        esl = slice(es * 512, (es + 1) * 512)
                        ps = wops.tile([HD, 512], F32, tag="wo")
                        for h in range(HPC):
                            nc.tensor.matmul(
                                ps[:],
                                ctx_sb[:, h, tsl],
                                wo_sb[:, h, esl],
                                start=(h == 0),
                                stop=(h == HPC - 1),
                            )
                        if es % 2 == 0:
                            nc.scalar.copy(ev[:, esl], ps[:])
                        else:
                            nc.vector.tensor_copy(ev[:, esl], ps[:])
                    q = nc.sync if (2 * j + tck) % 2 == 0 else nc.gpsimd
                    q.dma_start(out_d[tsl, :], ev[:])

            def emit_ol(qinfo, st):
                # deferred p@v + l matmuls for an exp'd quad.  o and l
                # share one PSUM bank as ONE interleaved accumulation
                # group: the first pv matmul opens it (bank-wide
                # has_written clear), every later pv/l matmul
                # accumulates element-wise in its own column range.
                quad, lq = qinfo
                h = st["h"]

                def flags():
                    i = st["mm_i"]
                    st["mm_i"] += 1
                    return i == 0, i == st["nmm"] - 1

                for s, b in quad:
                    c = b["c"]
                    vsl = slice(h * HD, (h + 1) * HD)
                    sa, so = flags()
                    if b["kind"] == "full":
                        nc.tensor.matmul(
                            st["o"], v_all[:, c, vsl], st["pt"][:, s, :],
                            start=sa, stop=so, skip_group_check=True,
                        )
                    else:
                        nc.tensor.matmul(
                            st["o"][:, HD:PAIR], v_all[:, c, vsl],
                            st["pt"][:, s, 0:HD],
                            start=sa, stop=so, skip_group_check=True,
                        )
                fulls = [s for s, b in quad if b["kind"] == "full"]
                if lq is not None:
                    sa, so = flags()
                    nc.tensor.matmul(
                        st["l"], ones_sb[:], lq[:],
                        start=sa, stop=so, skip_group_check=True,
                    )
                elif len(fulls) == 1:
                    sa, so = flags()
                    nc.tensor.matmul(
                        st["l"], ones_sb[:], st["pt"][:, fulls[0], :],
                        start=sa, stop=so, skip_group_check=True,
                    )
                for s, b in quad:
                    # narrow blocks are folded into lq on the DVE; the
                    # PE fallback only fires for all-narrow quads
                    if b["kind"] == "narrow" and lq is None:
                        sa, so = flags()
                        nc.tensor.matmul(
                            st["l"][:, HD:PAIR], ones_sb[:],
                            st["pt"][:, s, 0:HD],
                            start=sa, stop=so, skip_group_check=True,
                        )
                return st["mm_i"] == st["nmm"]

            # pv/l matmuls lag up to TWO quads behind the score/exp
            # stream: at group boundaries the deferred work pads the PE
            # while the last quad's exp->fold chain completes.
            pending = []          # FIFO of ((quad, lq), st)
            fin = [None]

            def maybe_finalize(cur_st):
                fs = fin[0]
                if fs is not None and fs is not cur_st:
                    finalize(fs)
                    fin[0] = None
                    if fs["h"] == HPC - 1:
                        emit_outproj(fs["j"])

            def flush_one(cur_st):
                qinfo, pst = pending.pop(0)
                if emit_ol(qinfo, pst):
                    fin[0] = pst
                maybe_finalize(cur_st)

            prev_st = None
            # big groups first; the tiny groups (j=0,1) are slotted
            # mid-stream so their exp->fold->l latency chains hide
            # behind neighbouring groups' matmul streams
            for j in (6, 5, 4, 3, 1, 0, 2, 7):
                qsl = slice(j * PAIR, (j + 1) * PAIR)
                blocks = statuses[j]
                n = len(blocks)
                quads = [list(enumerate(blocks))[i:i + 4] for i in range(0, n, 4)]
                nmm = 0
                for quad in quads:
                    nf = sum(1 for _, b in quad if b["kind"] == "full")
                    nn = len(quad) - nf
                    nmm += len(quad) + (1 if nf >= 1 else nn)
                for h in range(HPC):
                    # only the previous group may still have deferred
                    # quads once this group's PSUM tile is allocated
                    # (olps has 2 buffers)
                    while pending and pending[0][1] is not prev_st:
                        flush_one(None)
                    ol_ps = olps.tile([HD, 2 * PAIR], F32, tag="ol")
                    pt = ptp.tile([HD, NCHUNK, PAIR], BF16, tag="pt")
                    st = {"o": ol_ps[:, 0:PAIR], "l": ol_ps[:, PAIR:2 * PAIR],
                          "pt": pt, "h": h, "qsl": qsl, "j": j,
                          "mm_i": 0, "nmm": nmm}
                    for qi, quad in enumerate(quads):
                        s_ps = sps.tile([HD, 4, PAIR], F32, tag="s")
                        for s, b in quad:
                            c = b["c"]
                            ksl = slice(c * HD, (c + 1) * HD)
                            sl = s - qi * 4
                            if b["kind"] == "full":
                                nc.tensor.matmul(
                                    s_ps[:, sl, :],
                                    qk_sb[:, HPC + h, ksl],
                                    qk_sb[:, h, qsl],
                                    start=True, stop=True,
                                )
                            else:
                                nc.tensor.matmul(
                                    s_ps[:, sl, 0:HD],
                                    qk_sb[:, HPC + h, ksl],
                                    qk_sb[:, h, j * PAIR + HD:(j + 1) * PAIR],
                                    start=True, stop=True,
                                )
                        # exp: ONE ACT op per quad, full-width.  A narrow
                        # slot's upper 128 columns hold PSUM garbage whose
                        # exp lands in pt columns no consumer ever reads
                        # (pv/l/mask touch only its 0:128 range).
                        nc.scalar.activation(
                            pt[:, qi * 4:qi * 4 + len(quad), :],
                            s_ps[:, 0:len(quad), :], EXP,
                        )
                        # multiplicative exp-mask per query half
                        # (exp(s+m) == exp(s)*exp(m)), off the exp chain
                        for s, b in quad:
                            if b["kind"] == "full":
                                if b["lo"] >= 0:
                                    nc.vector.tensor_mul(
                                        pt[:, s, 0:HD], pt[:, s, 0:HD],
                                        mk_sb[:, b["lo"], :])
                                if b["hi"] >= 0:
                                    nc.vector.tensor_mul(
                                        pt[:, s, HD:PAIR], pt[:, s, HD:PAIR],
                                        mk_sb[:, b["hi"], :])
                            else:
                                if b["hi"] >= 0:
                                    nc.vector.tensor_mul(
                                        pt[:, s, 0:HD], pt[:, s, 0:HD],
                                        mk_sb[:, b["hi"], :])
                        # DVE quad-fold of full blocks for the l matmul;
                        # narrow blocks fold their 128 columns into the
                        # upper half of lq (sparing the PE an l matmul)
                        lq = None
                        nf = len([1 for _, b in quad if b["kind"] == "full"])
                        narrows = [s for s, b in quad if b["kind"] == "narrow"]
                        s0 = quad[0][0]
                        if nf == 4:
                            # full quads sit early in the group (not on
                            # the boundary-critical chain): their fold
                            # tree runs on the otherwise-idle GpSimd,
                            # unloading the near-saturated DVE
                            t2 = lqp.tile([HD, 2, PAIR], BF16, tag="t2")
                            nc.gpsimd.tensor_add(
                                t2[:], pt[:, s0:s0 + 2, :], pt[:, s0 + 2:s0 + 4, :])
                            lq = lqp.tile([HD, PAIR], BF16, tag="lq")
                            nc.gpsimd.tensor_add(lq[:], t2[:, 0, :], t2[:, 1, :])
                        elif nf == 3:
                            lq = lqp.tile([HD, PAIR], BF16, tag="lq")
                            nc.vector.tensor_add(
                                lq[:], pt[:, s0, :], pt[:, s0 + 1, :])
                            nc.vector.tensor_add(lq[:], lq[:], pt[:, s0 + 2, :])
                        elif nf == 2:
                            lq = lqp.tile([HD, PAIR], BF16, tag="lq")
                            nc.vector.tensor_add(
                                lq[:], pt[:, s0, :], pt[:, s0 + 1, :])
                        elif nf == 1 and narrows:
                            lq = lqp.tile([HD, PAIR], BF16, tag="lq")
                            nc.vector.tensor_copy(lq[:], pt[:, s0, :])
                        if lq is not None:
                            for s in narrows:
                                nc.vector.tensor_add(
                                    lq[:, HD:PAIR], lq[:, HD:PAIR],
                                    pt[:, s, 0:HD])
                        # queue this quad's pv/l; flush the oldest once
                        # more than two are deferred
                        pending.append(((quad, lq), st))
                        while len(pending) > 3:
                            flush_one(st)
                        maybe_finalize(st)
                    prev_st = st
            while pending:
                flush_one(None)
            maybe_finalize(None)
    nc.compile()
    return nc


_PERM = np.concatenate(
    [np.concatenate([np.arange(0, HD, 2), np.arange(1, HD, 2)]) + h * HD
     for h in range(HPC)]
)


def prepare(x, freqs, mask, wq, wk, wv, wo):
    """Host-side sharding/prep. Returns (nc, in_maps)."""
    x = np.asarray(x, np.float32)
    freqs = np.asarray(freqs, np.float32)
    mask = np.asarray(mask, np.float32)
    wq, wk, wv, wo = (np.asarray(w, np.float32) for w in (wq, wk, wv, wo))

    statuses, maskt = _mask_structure(mask)
    nc = _build_program(statuses, maskt.shape[1])

    scale = np.float32(1.0 / np.sqrt(HD))
    cos = np.ascontiguousarray(freqs[:, :, 0].T)  # (64, T)
    sin = np.ascontiguousarray(freqs[:, :, 1].T)
    cs = np.empty((HD, 2, T), np.float32)
    cs[0:64, 0, :] = cos
    cs[64:128, 0, :] = cos
    cs[0:64, 1, :] = -sin
    cs[64:128, 1, :] = sin
    # per-slice layout so each slice's DMA is one contiguous
    # 2KB-per-partition run
    cs = np.ascontiguousarray(
        cs.reshape(HD, 2, NSLICE, PROJ).transpose(2, 0, 1, 3)
    ).astype(NPBF16)

    ones_sq = np.ones((HD, HD), NPBF16)
    maskt_bf = maskt.astype(NPBF16)

    def pshuf_w(w):
        # [D, E] -> [HD, NCHUNK, E]: partition-major so each SBUF
        # partition's content is one contiguous DRAM run
        return np.ascontiguousarray(
            w.reshape(NCHUNK, HD, w.shape[1]).transpose(1, 0, 2)
        ).astype(NPBF16)

    def pshuf_x(xb):
        # x[b].T [D, T] -> [NSLICE, HD, NCHUNK, PROJ]
        xT = xb.T.reshape(NCHUNK, HD, NSLICE, PROJ)
        return np.ascontiguousarray(xT.transpose(2, 1, 0, 3)).astype(NPBF16)

    xts = [pshuf_x(x[b]) for b in range(B)]

    in_maps = []
    for core in range(8):
        b, g = core // 4, core % 4
        cols = slice(g * HPC * HD, (g + 1) * HPC * HD)
        wot = wo.T[cols, :].reshape(HPC, HD, D).transpose(1, 0, 2)
        in_maps.append({
            "xts": xts[b],
            "wqt": pshuf_w((wq.T[:, cols] * scale)[:, _PERM]),
            "wkt": pshuf_w(wk.T[:, cols][:, _PERM]),
            "wvt": pshuf_w(wv.T[:, cols]),
            "wot": np.ascontiguousarray(wot).astype(NPBF16),
            "cs": cs,
            "maskt": maskt_bf,
            "ones_sq": ones_sq,
        })
    return nc, in_maps


def run(x, freqs, mask, wq, wk, wv, wo, **spmd_kwargs):
    nc, in_maps = prepare(x, freqs, mask, wq, wk, wv, wo)
    res = run_bass_kernel_spmd(nc, in_maps, list(range(8)), **spmd_kwargs)
    parts = [np.asarray(res.results[c]["out"], np.float32) for c in range(8)]
    out = np.stack([
        parts[b * 4] + parts[b * 4 + 1] + parts[b * 4 + 2] + parts[b * 4 + 3]
        for b in range(B)
    ])
    return out, res


def kernel(x, freqs, mask, wq, wk, wv, wo):
    out, _ = run(x, freqs, mask, wq, wk, wv, wo)
    return out


# revision 57
# speedup vs baseline: 1.3017x; 1.3017x over previous
"""Trainium2 Bass kernel for causal self-attention with RoPE.

Shapes: x (2, 2048, 2048), 16 heads x 128 head_dim.
Sharding: 8 cores = 2 batch x 4 head-groups (4 heads per core).
Each core computes q/k/v projections for its heads, RoPE, causal-masked
softmax attention, and a partial output projection (its head columns of
wo); the host sums the 4 partials per batch element.

Layout strategy (per core):
  - all matmul operands in bf16; accumulation fp32 in PSUM.
  - q,k built in transposed layout (head_dim on partitions, t free) so
    RoPE and the score matmuls need no on-device transposes.
  - the v projection is interleaved into the q/k pass per 512-query
    slice (x^T is streamed from HBM exactly once).
  - scores computed as s^T (keys x q) per 256-query group.  The causal
    structure is exploited at query-HALF granularity: for the diagonal
    key chunk whose lower query half is fully masked, only a 128-wide
    matmul is issued ("narrow" block).
  - softmax denominators: instead of one ones-matmul per key block
    (which costs as much PE streaming as p@v itself), pt blocks are
    pre-folded in quads on the DVE and a single 256-wide ones-matmul
    per quad accumulates l in PSUM.  o and l live in one PSUM bank as
    a single interleaved accumulation group (pv opens it; all later
    pv/l matmuls accumulate element-wise).
  - p@v and l matmuls trail the score/exp stream by up to three quads
    so group boundaries have deferred PE work to hide the exp latency;
    big query groups bracket the stream (tiny j=0/1 mid-stream, j=7
    last so the final drain has padding work).
  - output projection per query group is interleaved right after its
    last head finalizes; PSUM evacuations alternate ScalarE/VectorE
    and the output is written in bf16 (host sums partials in fp32).
  - startup: dummy matmuls on a memset tile warm the PE clock (HAM)
    and bridge the DMA wait; wq/xt arrive in need-order pieces, each
    critical piece ordered so no big-descriptor transfer competes
    with it (DMA queues are FIFO; bus share scales with descriptor
    size = piece bytes per partition).
"""

import sys
from contextlib import ExitStack

if "/opt/trn_rl_repo" not in sys.path:
    sys.path.insert(0, "/opt/trn_rl_repo")

import numpy as np
import ml_dtypes

import concourse.bacc as bacc
import concourse.mybir as mybir
import concourse.tile as tile
from concourse.bass_utils import run_bass_kernel_spmd

B, T, D, NH, HD = 2, 2048, 2048, 16, 128
HPC = 4              # heads per core
PAIR = 256           # queries per attention group
NPAIR = T // PAIR    # 8
NCHUNK = T // HD     # 16 key chunks of 128
PROJ = 512           # queries per projection slice
NSLICE = T // PROJ   # 4
BF16 = mybir.dt.bfloat16
F32 = mybir.dt.float32
NPBF16 = np.dtype(ml_dtypes.bfloat16)


def _mask_structure(mask):
    """Classify (query-group, key-chunk) blocks at query-half granularity.

    Returns (statuses, maskt):
      statuses[j] = list of blocks, full-width blocks first (ascending
        chunk), then narrow blocks.  Each block is a dict:
          {"c": chunk, "kind": "full"|"narrow", "lo": mi, "hi": mi}
        where mi is a mask-tile index or -1 (half fully visible).
        "narrow" means the lower query half is fully masked; only the
        upper 128 queries are computed.
      maskt: packed (128, nmask, 128) array of deduplicated transposed
        exp-mask tiles ([keys, queries]).
    """
    statuses = []
    tiles = {}
    tile_list = []

    def tile_idx(sub):
        key = sub.tobytes()
        mi = tiles.get(key)
        if mi is None:
            assert np.all(sub <= 64.0), "additive mask too large for exp-mask"
            mi = len(tile_list)
            tiles[key] = mi
            tile_list.append(np.exp(np.ascontiguousarray(sub.T)))
        return mi

    def half_state(h):
        if np.all(h <= -1e8):
            return "skip"
        if np.all(h == 0.0):
            return "free"
        return "part"

    for j in range(NPAIR):
        full, narrow = [], []
        for c in range(NCHUNK):
            ks = slice(c * HD, (c + 1) * HD)
            lo = mask[j * PAIR:j * PAIR + HD, ks]
            hi = mask[j * PAIR + HD:(j + 1) * PAIR, ks]
            slo, shi = half_state(lo), half_state(hi)
            if slo == "skip" and shi == "skip":
                continue
            if slo == "skip":
                narrow.append({"c": c, "kind": "narrow", "lo": -1,
                               "hi": -1 if shi == "free" else tile_idx(hi)})
            else:
                full.append({"c": c, "kind": "full",
                             "lo": -1 if slo == "free" else tile_idx(lo),
                             "hi": -1 if shi == "free" else tile_idx(hi)})
        if not full:
            # pv group opener must cover all 256 o columns
            b = narrow.pop(0)
            c = b["c"]
            full.append({"c": c, "kind": "full",
                         "lo": tile_idx(mask[j * PAIR:j * PAIR + HD,
                                             c * HD:(c + 1) * HD]),
                         "hi": b["hi"]})
        assert full or narrow, f"query group {j} fully masked"
        statuses.append(full + narrow)
    nmask = max(1, len(tile_list))
    maskt = np.zeros((HD, nmask, HD), np.float32)
    for i, t in enumerate(tile_list):
        maskt[:, i, :] = t
    return statuses, maskt


def _build_program(statuses, nmask):
    nc = bacc.Bacc(None, target_bir_lowering=False)

    # All inputs are laid out on the host so that each SBUF partition's
    # content is one contiguous DRAM run (large descriptors keep the
    # DMA queues at full bus rate).
    xt_d = nc.dram_tensor("xts", [NSLICE, HD, NCHUNK, PROJ], BF16,
                          kind="ExternalInput")
    wq_d = nc.dram_tensor("wqt", [HD, NCHUNK, HPC * HD], BF16, kind="ExternalInput")
    wk_d = nc.dram_tensor("wkt", [HD, NCHUNK, HPC * HD], BF16, kind="ExternalInput")
    wv_d = nc.dram_tensor("wvt", [HD, NCHUNK, HPC * HD], BF16, kind="ExternalInput")
    wo_d = nc.dram_tensor("wot", [HD, HPC, D], BF16, kind="ExternalInput")
    cs_d = nc.dram_tensor("cs", [NSLICE, HD, 2, PROJ], BF16, kind="ExternalInput")
    mk_d = nc.dram_tensor("maskt", [HD, nmask, HD], BF16, kind="ExternalInput")
    ones_d = nc.dram_tensor("ones_sq", [HD, HD], BF16, kind="ExternalInput")
    out_d = nc.dram_tensor("out", [T, D], BF16, kind="ExternalOutput")

    xt_ap = xt_d.ap()
    wq_ap = wq_d.ap()
    wk_ap = wk_d.ap()
    wv_ap = wv_d.ap()
    wo_ap = wo_d.ap()
    EXP = mybir.ActivationFunctionType.Exp

    with tile.TileContext(nc) as tc, ExitStack() as top:
        # ---- PE warmup: dummy matmuls while the first DMAs land ----
        # The HAM clock gate holds the PE at 1.2 GHz until it has seen
        # ~3.4us of sustained activity; these no-dependency matmuls
        # burn that window during the DMA wait so the first real
        # matmuls run at 2.4 GHz.
        constp = top.enter_context(tc.tile_pool(name="const", bufs=1))
        ones_sb = constp.tile([HD, HD], BF16)
        wsrc = constp.tile([HD, HD], BF16)
        nc.gpsimd.memset(wsrc[:], 0.0)
        with ExitStack() as wh:
            wpp = wh.enter_context(tc.tile_pool(name="wpp", bufs=1, space="PSUM"))
            wps = wpp.tile([HD, HD], F32)
            for _ in range(52):
                nc.tensor.matmul(wps[:], wsrc[:], wsrc[:], start=True, stop=True)

        qkp = top.enter_context(tc.tile_pool(name="qkp", bufs=1))
        # q heads at [:, h, :], k heads at [:, 4+h, :]
        qk_sb = qkp.tile([HD, 2 * HPC, T], BF16)
        vap = top.enter_context(tc.tile_pool(name="vap", bufs=1))
        v_all = vap.tile([HD, NCHUNK, HPC * HD], BF16)
        ctxp = top.enter_context(tc.tile_pool(name="ctxp", bufs=1))
        ctx_sb = ctxp.tile([HD, HPC, T], BF16)
        wop = top.enter_context(tc.tile_pool(name="wop", bufs=1))
        wo_sb = wop.tile([HD, HPC, D], BF16)
        mkpre = top.enter_context(tc.tile_pool(name="mkpre", bufs=1))
        mk_sb = mkpre.tile([HD, nmask, HD], BF16)

        # ---- combined q/k/v projection pass (+ fused RoPE) ----
        with ExitStack() as ph:
            wp = ph.enter_context(tc.tile_pool(name="wp", side="right", bufs=1))
            wvp = ph.enter_context(tc.tile_pool(name="wvp", side="right", bufs=1))
            xtp = ph.enter_context(tc.tile_pool(name="xtp", side="right", bufs=2))
            csp = ph.enter_context(tc.tile_pool(name="csp", side="right", bufs=2))
            ropep = ph.enter_context(tc.tile_pool(name="ropep", side="right", bufs=2))
            pps = ph.enter_context(tc.tile_pool(name="pps", bufs=6, space="PSUM"))
            vps = ph.enter_context(tc.tile_pool(name="vps", bufs=2, space="PSUM"))
            wqk_sb = wp.tile([HD, 2, NCHUNK, HPC * HD], BF16)
            wv_sb = wvp.tile([HD, NCHUNK, HPC * HD], BF16)

            # Startup choreography: the bus is saturated for the first
            # ~40us; per-queue rate scales with descriptor size
            # (bytes-per-partition of the piece), and a queue round-robins
            # batches across ALL its queued transfers, so a critical
            # piece finishes promptly only if nothing is queued behind it.
            # Each need-ordered piece therefore rides its own queue
            # (scalar/gpsimd for weights, sync/vector for activations).
            nc.scalar.dma_start(wqk_sb[:, 0, 0:2, :], wq_ap[:, 0:2, :])
            nc.scalar.dma_start(wqk_sb[:, 0, 2:8, :], wq_ap[:, 2:8, :])
            nc.scalar.dma_start(wqk_sb[:, 0, 8:, :], wq_ap[:, 8:, :])
            cs0 = csp.tile([HD, 2, PROJ], BF16, tag="cs")
            nc.gpsimd.dma_start(cs0[:], cs_d.ap()[0])
            nc.scalar.dma_start(wqk_sb[:, 1, 0:4, :], wk_ap[:, 0:4, :])
            nc.scalar.dma_start(wv_sb[:], wv_ap[:])

            for ns in range(NSLICE):
                tsl = slice(ns * PROJ, (ns + 1) * PROJ)
                xt = xtp.tile([HD, NCHUNK, PROJ], BF16, tag="xt")
                if ns == 0:
                    cs_sl = cs0
                    nc.sync.dma_start(xt[:, 0:2, :], xt_ap[ns, :, 0:2, :])
                    nc.sync.dma_start(xt[:, 2:8, :], xt_ap[ns, :, 2:8, :])
                    nc.sync.dma_start(xt[:, 8:, :], xt_ap[ns, :, 8:, :])
                    # the wk tail follows the critical x pieces on sync
                    # (its big descriptors would otherwise starve them);
                    # the first quarter rides scalar so the k
                    # accumulation can start early
                    nc.sync.dma_start(wqk_sb[:, 1, 4:, :], wk_ap[:, 4:, :])
                else:
                    cs_sl = csp.tile([HD, 2, PROJ], BF16, tag="cs")
                    nc.sync.dma_start(xt[:], xt_ap[ns])
                    nc.scalar.dma_start(cs_sl[:], cs_d.ap()[ns])
                if ns == 2:
                    # tiny attention constants go late: their 256B
                    # descriptors would eat DMA-engine slots during the
                    # startup-critical window
                    nc.gpsimd.dma_start(ones_sb[:], ones_d[:])
                    nc.gpsimd.dma_start(mk_sb[:], mk_d[:])
                if ns == 1:
                    # wo rides the sync queue once the startup-critical
                    # pieces are through (needed only by the first
                    # output projection, ~170us in)
                    nc.sync.dma_start(wo_sb[:], wo_ap[:])

                def rope(ps, wsel, h):
                    # RoPE: dst = raw*C + swap(raw)*S.  The swap is
                    # materialized by two ScalarE half-copies; VectorE
                    # does two multiplies and one add per tile.
                    dst = qk_sb[:, wsel * HPC + h, tsl]
                    sw = ropep.tile([HD, PROJ], F32, tag="sw")
                    nc.scalar.copy(sw[0:64, :], ps[64:128, :])
                    nc.scalar.copy(sw[64:128, :], ps[0:64, :])
                    tb = ropep.tile([HD, PROJ], F32, tag="tb")
                    nc.vector.tensor_mul(dst, ps[:], cs_sl[:, 0, :])
                    nc.vector.tensor_mul(tb[:], sw[:], cs_sl[:, 1, :])
                    nc.vector.tensor_add(dst, dst, tb[:])

                if ns == 0:
                    # startup: q accumulations run in chunk phases that
                    # match the wq/xt DMA pieces, all heads per phase
                    qtiles = [pps.tile([HD, PROJ], F32, tag="ps",
                                       name=f"qps{h}")
                              for h in range(HPC)]
                    wrm = pps.tile([HD, PROJ], F32, tag="ps", name="wrm")
                    for a, b in ((0, 2), (2, 8), (8, NCHUNK)):
                        for h in range(HPC):
                            hs = slice(h * HD, (h + 1) * HD)
                            for k in range(a, b):
                                nc.tensor.matmul(
                                    qtiles[h][:], wqk_sb[:, 0, k, hs],
                                    xt[:, k, :],
                                    start=(k == 0), stop=(k == NCHUNK - 1),
                                )
                            if b == NCHUNK:
                                rope(qtiles[h], 0, h)
                        if b == 2:
                            # keep the PE busy (and the HAM clock warm)
                            # while the next wq/xt pieces land
                            for _ in range(52):
                                nc.tensor.matmul(wrm[:, 0:HD], wsrc[:],
                                                 wsrc[:], start=True,
                                                 stop=True)
                    ktiles = [pps.tile([HD, PROJ], F32, tag="ps",
                                       name=f"kps{h}")
                              for h in range(HPC)]
                    for a, b in ((0, 4), (4, NCHUNK)):
                        for h in range(HPC):
                            hs = slice(h * HD, (h + 1) * HD)
                            for k in range(a, b):
                                nc.tensor.matmul(
                                    ktiles[h][:], wqk_sb[:, 1, k, hs],
                                    xt[:, k, :],
                                    start=(k == 0), stop=(k == NCHUNK - 1),
                                )
                            if b == NCHUNK:
                                rope(ktiles[h], 1, h)
                else:
                    for wsel in range(2):
                        for h in range(HPC):
                            ps = pps.tile([HD, PROJ], F32, tag="ps")
                            hs = slice(h * HD, (h + 1) * HD)
                            for k in range(NCHUNK):
                                nc.tensor.matmul(
                                    ps[:],
                                    wqk_sb[:, wsel, k, hs],
                                    xt[:, k, :],
                                    start=(k == 0),
                                    stop=(k == NCHUNK - 1),
                                )
                            rope(ps, wsel, h)

                # ---- v projection for this slice (x^T stationary) ----
                for tc2 in range(PROJ // HD):
                    vp_t = vps.tile([HD, HPC * HD], F32, tag="vps")
                    for k in range(NCHUNK):
                        nc.tensor.matmul(
                            vp_t[:],
                            xt[:, k, tc2 * HD:(tc2 + 1) * HD],
                            wv_sb[:, k, :],
                            start=(k == 0),
                            stop=(k == NCHUNK - 1),
                        )
                    nc.scalar.copy(v_all[:, ns * (PROJ // HD) + tc2, :], vp_t[:])

        # ---- attention + interleaved output projection ----
        with ExitStack() as ph:
            ptp = ph.enter_context(tc.tile_pool(name="ptp", side="right", bufs=2))
            lrp = ph.enter_context(tc.tile_pool(name="lrp", side="right", bufs=2))
            lqp = ph.enter_context(tc.tile_pool(name="lqp", side="right", bufs=4))
            evp = ph.enter_context(tc.tile_pool(name="evp", side="right", bufs=2))
            sps = ph.enter_context(tc.tile_pool(name="sps", bufs=2, space="PSUM"))
            olps = ph.enter_context(tc.tile_pool(name="olps", bufs=2, space="PSUM"))
            wops = ph.enter_context(tc.tile_pool(name="wops", bufs=2, space="PSUM"))

            def finalize(st):
                # DVE fast-recip of the (already partition-broadcast) row
                # sums, then one DVE multiply into ctx
                lr = lrp.tile([HD, PAIR], F32, tag="lr")
                nc.vector.reciprocal_approx_fast(lr[:], st["l"])
                nc.vector.tensor_mul(
                    ctx_sb[:, st["h"], st["qsl"]], st["o"], lr[:]
                )

            def emit_outproj(j):
                # output projection for query group j (all heads final);
                # PSUM evacuations alternate ScalarE/VectorE, output DMA
                # is one 512KB transfer per 128-query tile, queues
                # alternating sync/gpsimd.
                for tck in range(PAIR // HD):
                    tsl = slice(j * PAIR + tck * HD, j * PAIR + (tck + 1) * HD)
                    ev = evp.tile([HD, D], BF16, tag="ev")
                    for es in range(D // 512):
                        esl = slice(es * 512, (es + 1) * 512)
                        ps = wops.tile([HD, 512], F32, tag="wo")
                        for h in range(HPC):
                            nc.tensor.matmul(
                                ps[:],
                                ctx_sb[:, h, tsl],
                                wo_sb[:, h, esl],
                                start=(h == 0),
                                stop=(h == HPC - 1),
                            )
                        if es % 2 == 0:
                            nc.scalar.copy(ev[:, esl], ps[:])
                        else:
                            nc.vector.tensor_copy(ev[:, esl], ps[:])
                    q = nc.sync if (2 * j + tck) % 2 == 0 else nc.gpsimd
                    q.dma_start(out_d[tsl, :], ev[:])

            def emit_ol(qinfo, st):
                # deferred p@v + l matmuls for an exp'd quad.  o and l
                # share one PSUM bank as ONE interleaved accumulation
                # group: the first pv matmul opens it (bank-wide
                # has_written clear), every later pv/l matmul
                # accumulates element-wise in its own column range.
                quad, lq = qinfo
                h = st["h"]

                def flags():
                    i = st["mm_i"]
                    st["mm_i"] += 1
                    return i == 0, i == st["nmm"] - 1

                for s, b in quad:
                    c = b["c"]
                    vsl = slice(h * HD, (h + 1) * HD)
                    sa, so = flags()
                    if b["kind"] == "full":
                        nc.tensor.matmul(
                            st["o"], v_all[:, c, vsl], st["pt"][:, s, :],
                            start=sa, stop=so, skip_group_check=True,
                        )
                    else:
                        nc.tensor.matmul(
                            st["o"][:, HD:PAIR], v_all[:, c, vsl],
                            st["pt"][:, s, 0:HD],
                            start=sa, stop=so, skip_group_check=True,
                        )
                fulls = [s for s, b in quad if b["kind"] == "full"]
                if lq is not None:
                    sa, so = flags()
                    nc.tensor.matmul(
                        st["l"], ones_sb[:], lq[:],
                        start=sa, stop=so, skip_group_check=True,
                    )
                elif len(fulls) == 1:
                    sa, so = flags()
                    nc.tensor.matmul(
                        st["l"], ones_sb[:], st["pt"][:, fulls[0], :],
                        start=sa, stop=so, skip_group_check=True,
                    )
                for s, b in quad:
                    # narrow blocks are folded into lq on the DVE; the
                    # PE fallback only fires for all-narrow quads
                    if b["kind"] == "narrow" and lq is None:
                        sa, so = flags()
                        nc.tensor.matmul(
                            st["l"][:, HD:PAIR], ones_sb[:],
                            st["pt"][:, s, 0:HD],
                            start=sa, stop=so, skip_group_check=True,
                        )
                return st["mm_i"] == st["nmm"]

            # pv/l matmuls lag up to TWO quads behind the score/exp
            # stream: at group boundaries the deferred work pads the PE
            # while the last quad's exp->fold chain completes.
            pending = []          # FIFO of ((quad, lq), st)
            fin = [None]

            def maybe_finalize(cur_st):
                fs = fin[0]
                if fs is not None and fs is not cur_st:
                    finalize(fs)
                    fin[0] = None
                    if fs["h"] == HPC - 1:
                        emit_outproj(fs["j"])

            def flush_one(cur_st):
                qinfo, pst = pending.pop(0)
                if emit_ol(qinfo, pst):
                    fin[0] = pst
                maybe_finalize(cur_st)

            prev_st = None
            # big groups first; the tiny groups (j=0,1) are slotted
            # mid-stream so their exp->fold->l latency chains hide
            # behind neighbouring groups' matmul streams
            for j in (6, 5, 4, 3, 1, 0, 2, 7):
                qsl = slice(j * PAIR, (j + 1) * PAIR)
                blocks = statuses[j]
                n = len(blocks)
                quads = [list(enumerate(blocks))[i:i + 4] for i in range(0, n, 4)]
                nmm = 0
                for quad in quads:
                    nf = sum(1 for _, b in quad if b["kind"] == "full")
                    nn = len(quad) - nf
                    nmm += len(quad) + (1 if nf >= 1 else nn)
                for h in range(HPC):
                    # only the previous group may still have deferred
                    # quads once this group's PSUM tile is allocated
                    # (olps has 2 buffers)
                    while pending and pending[0][1] is not prev_st:
                        flush_one(None)
                    ol_ps = olps.tile([HD, 2 * PAIR], F32, tag="ol")
                    pt = ptp.tile([HD, NCHUNK, PAIR], BF16, tag="pt")
                    st = {"o": ol_ps[:, 0:PAIR], "l": ol_ps[:, PAIR:2 * PAIR],
                          "pt": pt, "h": h, "qsl": qsl, "j": j,
                          "mm_i": 0, "nmm": nmm}
                    for qi, quad in enumerate(quads):
                        s_ps = sps.tile([HD, 4, PAIR], F32, tag="s")
                        for s, b in quad:
                            c = b["c"]
                            ksl = slice(c * HD, (c + 1) * HD)
                            sl = s - qi * 4
                            if b["kind"] == "full":
                                nc.tensor.matmul(
                                    s_ps[:, sl, :],
                                    qk_sb[:, HPC + h, ksl],
                                    qk_sb[:, h, qsl],
                                    start=True, stop=True,
                                )
                            else:
                                nc.tensor.matmul(
                                    s_ps[:, sl, 0:HD],
                                    qk_sb[:, HPC + h, ksl],
                                    qk_sb[:, h, j * PAIR + HD:(j + 1) * PAIR],
                                    start=True, stop=True,
                                )
                        # exp: ONE ACT op per quad, full-width.  A narrow
                        # slot's upper 128 columns hold PSUM garbage whose
                        # exp lands in pt columns no consumer ever reads
                        # (pv/l/mask touch only its 0:128 range).
                        nc.scalar.activation(
                            pt[:, qi * 4:qi * 4 + len(quad), :],
                            s_ps[:, 0:len(quad), :], EXP,
                        )
                        # multiplicative exp-mask per query half
                        # (exp(s+m) == exp(s)*exp(m)), off the exp chain
                        for s, b in quad:
                            if b["kind"] == "full":
                                if b["lo"] >= 0:
                                    nc.vector.tensor_mul(
                                        pt[:, s, 0:HD], pt[:, s, 0:HD],
                                        mk_sb[:, b["lo"], :])
                                if b["hi"] >= 0:
                                    nc.vector.tensor_mul(
                                        pt[:, s, HD:PAIR], pt[:, s, HD:PAIR],
                                        mk_sb[:, b["hi"], :])
                            else:
                                if b["hi"] >= 0:
                                    nc.vector.tensor_mul(
                                        pt[:, s, 0:HD], pt[:, s, 0:HD],
                                        mk_sb[:, b["hi"], :])
                        # DVE quad-fold of full blocks for the l matmul;
                        # narrow blocks fold their 128 columns into the
                        # upper half of lq (sparing the PE an l matmul)
                        lq = None
                        nf = len([1 for _, b in quad if b["kind"] == "full"])
                        narrows = [s for s, b in quad if b["kind"] == "narrow"]
                        s0 = quad[0][0]
                        if nf == 4:
                            t2 = lqp.tile([HD, 2, PAIR], BF16, tag="t2")
                            nc.vector.tensor_add(
                                t2[:], pt[:, s0:s0 + 2, :], pt[:, s0 + 2:s0 + 4, :])
                            lq = lqp.tile([HD, PAIR], BF16, tag="lq")
                            nc.vector.tensor_add(lq[:], t2[:, 0, :], t2[:, 1, :])
                        elif nf == 3:
                            lq = lqp.tile([HD, PAIR], BF16, tag="lq")
                            nc.vector.tensor_add(
                                lq[:], pt[:, s0, :], pt[:, s0 + 1, :])
                            nc.vector.tensor_add(lq[:], lq[:], pt[:, s0 + 2, :])
                        elif nf == 2:
                            lq = lqp.tile([HD, PAIR], BF16, tag="lq")
                            nc.vector.tensor_add(
                                lq[:], pt[:, s0, :], pt[:, s0 + 1, :])
                        elif nf == 1 and narrows:
                            lq = lqp.tile([HD, PAIR], BF16, tag="lq")
                            nc.vector.tensor_copy(lq[:], pt[:, s0, :])
                        if lq is not None:
                            for s in narrows:
                                nc.vector.tensor_add(
                                    lq[:, HD:PAIR], lq[:, HD:PAIR],
                                    pt[:, s, 0:HD])
                        # queue this quad's pv/l; flush the oldest once
                        # more than two are deferred
                        pending.append(((quad, lq), st))
                        while len(pending) > 3:
                            flush_one(st)
                        maybe_finalize(st)
                    prev_st = st
            while pending:
                flush_one(None)
            maybe_finalize(None)
    nc.compile()
    return nc


_PERM = np.concatenate(
    [np.concatenate([np.arange(0, HD, 2), np.arange(1, HD, 2)]) + h * HD
     for h in range(HPC)]
)


def prepare(x, freqs, mask, wq, wk, wv, wo):
    """Host-side sharding/prep. Returns (nc, in_maps)."""
    x = np.asarray(x, np.float32)
    freqs = np.asarray(freqs, np.float32)
    mask = np.asarray(mask, np.float32)
    wq, wk, wv, wo = (np.asarray(w, np.float32) for w in (wq, wk, wv, wo))

    statuses, maskt = _mask_structure(mask)
    nc = _build_program(statuses, maskt.shape[1])

    scale = np.float32(1.0 / np.sqrt(HD))
    cos = np.ascontiguousarray(freqs[:, :, 0].T)  # (64, T)
    sin = np.ascontiguousarray(freqs[:, :, 1].T)
    cs = np.empty((HD, 2, T), np.float32)
    cs[0:64, 0, :] = cos
    cs[64:128, 0, :] = cos
    cs[0:64, 1, :] = -sin
    cs[64:128, 1, :] = sin
    # per-slice layout so each slice's DMA is one contiguous
    # 2KB-per-partition run
    cs = np.ascontiguousarray(
        cs.reshape(HD, 2, NSLICE, PROJ).transpose(2, 0, 1, 3)
    ).astype(NPBF16)

    ones_sq = np.ones((HD, HD), NPBF16)
    maskt_bf = maskt.astype(NPBF16)

    def pshuf_w(w):
        # [D, E] -> [HD, NCHUNK, E]: partition-major so each SBUF
        # partition's content is one contiguous DRAM run
        return np.ascontiguousarray(
            w.reshape(NCHUNK, HD, w.shape[1]).transpose(1, 0, 2)
        ).astype(NPBF16)

    def pshuf_x(xb):
        # x[b].T [D, T] -> [NSLICE, HD, NCHUNK, PROJ]
        xT = xb.T.reshape(NCHUNK, HD, NSLICE, PROJ)
        return np.ascontiguousarray(xT.transpose(2, 1, 0, 3)).astype(NPBF16)

    xts = [pshuf_x(x[b]) for b in range(B)]

    in_maps = []
    for core in range(8):
        b, g = core // 4, core % 4
        cols = slice(g * HPC * HD, (g + 1) * HPC * HD)
        wot = wo.T[cols, :].reshape(HPC, HD, D).transpose(1, 0, 2)
        in_maps.append({
            "xts": xts[b],
            "wqt": pshuf_w((wq.T[:, cols] * scale)[:, _PERM]),
            "wkt": pshuf_w(wk.T[:, cols][:, _PERM]),
            "wvt": pshuf_w(wv.T[:, cols]),
            "wot": np.ascontiguousarray(wot).astype(NPBF16),
            "cs": cs,
            "maskt": maskt_bf,
            "ones_sq": ones_sq,
        })
    return nc, in_maps


def run(x, freqs, mask, wq, wk, wv, wo, **spmd_kwargs):
    nc, in_maps = prepare(x, freqs, mask, wq, wk, wv, wo)
    res = run_bass_kernel_spmd(nc, in_maps, list(range(8)), **spmd_kwargs)
    parts = [np.asarray(res.results[c]["out"], np.float32) for c in range(8)]
    out = np.stack([
        parts[b * 4] + parts[b * 4 + 1] + parts[b * 4 + 2] + parts[b * 4 + 3]
        for b in range(B)
    ])
    return out, res


def kernel(x, freqs, mask, wq, wk, wv, wo):
    out, _ = run(x, freqs, mask, wq, wk, wv, wo)
    return out


# revision 58
# speedup vs baseline: 1.3081x; 1.0049x over previous
"""Trainium2 Bass kernel for causal self-attention with RoPE.

Shapes: x (2, 2048, 2048), 16 heads x 128 head_dim.
Sharding: 8 cores = 2 batch x 4 head-groups (4 heads per core).
Each core computes q/k/v projections for its heads, RoPE, causal-masked
softmax attention, and a partial output projection (its head columns of
wo); the host sums the 4 partials per batch element.

Layout strategy (per core):
  - all matmul operands in bf16; accumulation fp32 in PSUM.
  - q,k built in transposed layout (head_dim on partitions, t free) so
    RoPE and the score matmuls need no on-device transposes.
  - the v projection is interleaved into the q/k pass per 512-query
    slice (x^T is streamed from HBM exactly once).
  - scores computed as s^T (keys x q) per 256-query group.  The causal
    structure is exploited at query-HALF granularity: for the diagonal
    key chunk whose lower query half is fully masked, only a 128-wide
    matmul is issued ("narrow" block).
  - softmax denominators: instead of one ones-matmul per key block
    (which costs as much PE streaming as p@v itself), pt blocks are
    pre-folded in quads on the DVE and a single 256-wide ones-matmul
    per quad accumulates l in PSUM.  o and l live in one PSUM bank as
    a single interleaved accumulation group (pv opens it; all later
    pv/l matmuls accumulate element-wise).
  - p@v and l matmuls trail the score/exp stream by up to three quads
    so group boundaries have deferred PE work to hide the exp latency;
    big query groups bracket the stream (tiny j=0/1 mid-stream, j=7
    last so the final drain has padding work).
  - output projection per query group is interleaved right after its
    last head finalizes; PSUM evacuations alternate ScalarE/VectorE
    and the output is written in bf16 (host sums partials in fp32).
  - startup: dummy matmuls on a memset tile warm the PE clock (HAM)
    and bridge the DMA wait; wq/xt arrive in need-order pieces, each
    critical piece ordered so no big-descriptor transfer competes
    with it (DMA queues are FIFO; bus share scales with descriptor
    size = piece bytes per partition).
"""

import sys
from contextlib import ExitStack

if "/opt/trn_rl_repo" not in sys.path:
    sys.path.insert(0, "/opt/trn_rl_repo")

import numpy as np
import ml_dtypes

import concourse.bacc as bacc
import concourse.mybir as mybir
import concourse.tile as tile
from concourse.bass_utils import run_bass_kernel_spmd

B, T, D, NH, HD = 2, 2048, 2048, 16, 128
HPC = 4              # heads per core
PAIR = 256           # queries per attention group
NPAIR = T // PAIR    # 8
NCHUNK = T // HD     # 16 key chunks of 128
PROJ = 512           # queries per projection slice
NSLICE = T // PROJ   # 4
BF16 = mybir.dt.bfloat16
F32 = mybir.dt.float32
NPBF16 = np.dtype(ml_dtypes.bfloat16)


def _mask_structure(mask):
    """Classify (query-group, key-chunk) blocks at query-half granularity.

    Returns (statuses, maskt):
      statuses[j] = list of blocks, full-width blocks first (ascending
        chunk), then narrow blocks.  Each block is a dict:
          {"c": chunk, "kind": "full"|"narrow", "lo": mi, "hi": mi}
        where mi is a mask-tile index or -1 (half fully visible).
        "narrow" means the lower query half is fully masked; only the
        upper 128 queries are computed.
      maskt: packed (128, nmask, 128) array of deduplicated transposed
        exp-mask tiles ([keys, queries]).
    """
    statuses = []
    tiles = {}
    tile_list = []

    def tile_idx(sub):
        key = sub.tobytes()
        mi = tiles.get(key)
        if mi is None:
            assert np.all(sub <= 64.0), "additive mask too large for exp-mask"
            mi = len(tile_list)
            tiles[key] = mi
            tile_list.append(np.exp(np.ascontiguousarray(sub.T)))
        return mi

    def half_state(h):
        if np.all(h <= -1e8):
            return "skip"
        if np.all(h == 0.0):
            return "free"
        return "part"

    for j in range(NPAIR):
        full, narrow = [], []
        for c in range(NCHUNK):
            ks = slice(c * HD, (c + 1) * HD)
            lo = mask[j * PAIR:j * PAIR + HD, ks]
            hi = mask[j * PAIR + HD:(j + 1) * PAIR, ks]
            slo, shi = half_state(lo), half_state(hi)
            if slo == "skip" and shi == "skip":
                continue
            if slo == "skip":
                narrow.append({"c": c, "kind": "narrow", "lo": -1,
                               "hi": -1 if shi == "free" else tile_idx(hi)})
            else:
                full.append({"c": c, "kind": "full",
                             "lo": -1 if slo == "free" else tile_idx(lo),
                             "hi": -1 if shi == "free" else tile_idx(hi)})
        if not full:
            # pv group opener must cover all 256 o columns
            b = narrow.pop(0)
            c = b["c"]
            full.append({"c": c, "kind": "full",
                         "lo": tile_idx(mask[j * PAIR:j * PAIR + HD,
                                             c * HD:(c + 1) * HD]),
                         "hi": b["hi"]})
        assert full or narrow, f"query group {j} fully masked"
        statuses.append(full + narrow)
    nmask = max(1, len(tile_list))
    maskt = np.zeros((HD, nmask, HD), np.float32)
    for i, t in enumerate(tile_list):
        maskt[:, i, :] = t
    return statuses, maskt


def _build_program(statuses, nmask):
    nc = bacc.Bacc(None, target_bir_lowering=False)

    # All inputs are laid out on the host so that each SBUF partition's
    # content is one contiguous DRAM run (large descriptors keep the
    # DMA queues at full bus rate).
    xt_d = nc.dram_tensor("xts", [NSLICE, HD, NCHUNK, PROJ], BF16,
                          kind="ExternalInput")
    wq_d = nc.dram_tensor("wqt", [HD, NCHUNK, HPC * HD], BF16, kind="ExternalInput")
    wk_d = nc.dram_tensor("wkt", [HD, NCHUNK, HPC * HD], BF16, kind="ExternalInput")
    wv_d = nc.dram_tensor("wvt", [HD, NCHUNK, HPC * HD], BF16, kind="ExternalInput")
    wo_d = nc.dram_tensor("wot", [HD, HPC, D], BF16, kind="ExternalInput")
    cs_d = nc.dram_tensor("cs", [NSLICE, HD, 2, PROJ], BF16, kind="ExternalInput")
    mk_d = nc.dram_tensor("maskt", [HD, nmask, HD], BF16, kind="ExternalInput")
    ones_d = nc.dram_tensor("ones_sq", [HD, HD], BF16, kind="ExternalInput")
    out_d = nc.dram_tensor("out", [T, D], BF16, kind="ExternalOutput")

    xt_ap = xt_d.ap()
    wq_ap = wq_d.ap()
    wk_ap = wk_d.ap()
    wv_ap = wv_d.ap()
    wo_ap = wo_d.ap()
    EXP = mybir.ActivationFunctionType.Exp

    with tile.TileContext(nc) as tc, ExitStack() as top:
        # ---- PE warmup: dummy matmuls while the first DMAs land ----
        # The HAM clock gate holds the PE at 1.2 GHz until it has seen
        # ~3.4us of sustained activity; these no-dependency matmuls
        # burn that window during the DMA wait so the first real
        # matmuls run at 2.4 GHz.
        constp = top.enter_context(tc.tile_pool(name="const", bufs=1))
        ones_sb = constp.tile([HD, HD], BF16)
        wsrc = constp.tile([HD, HD], BF16)
        nc.gpsimd.memset(wsrc[:], 0.0)
        with ExitStack() as wh:
            wpp = wh.enter_context(tc.tile_pool(name="wpp", bufs=1, space="PSUM"))
            wps = wpp.tile([HD, HD], F32)
            for _ in range(52):
                nc.tensor.matmul(wps[:], wsrc[:], wsrc[:], start=True, stop=True)

        qkp = top.enter_context(tc.tile_pool(name="qkp", bufs=1))
        # q heads at [:, h, :], k heads at [:, 4+h, :]
        qk_sb = qkp.tile([HD, 2 * HPC, T], BF16)
        vap = top.enter_context(tc.tile_pool(name="vap", bufs=1))
        v_all = vap.tile([HD, NCHUNK, HPC * HD], BF16)
        ctxp = top.enter_context(tc.tile_pool(name="ctxp", bufs=1))
        ctx_sb = ctxp.tile([HD, HPC, T], BF16)
        wop = top.enter_context(tc.tile_pool(name="wop", bufs=1))
        wo_sb = wop.tile([HD, HPC, D], BF16)
        mkpre = top.enter_context(tc.tile_pool(name="mkpre", bufs=1))
        mk_sb = mkpre.tile([HD, nmask, HD], BF16)

        # ---- combined q/k/v projection pass (+ fused RoPE) ----
        with ExitStack() as ph:
            wp = ph.enter_context(tc.tile_pool(name="wp", side="right", bufs=1))
            wvp = ph.enter_context(tc.tile_pool(name="wvp", side="right", bufs=1))
            xtp = ph.enter_context(tc.tile_pool(name="xtp", side="right", bufs=2))
            csp = ph.enter_context(tc.tile_pool(name="csp", side="right", bufs=2))
            ropep = ph.enter_context(tc.tile_pool(name="ropep", side="right", bufs=2))
            pps = ph.enter_context(tc.tile_pool(name="pps", bufs=6, space="PSUM"))
            vps = ph.enter_context(tc.tile_pool(name="vps", bufs=2, space="PSUM"))
            wqk_sb = wp.tile([HD, 2, NCHUNK, HPC * HD], BF16)
            wv_sb = wvp.tile([HD, NCHUNK, HPC * HD], BF16)

            # Startup choreography: the bus is saturated for the first
            # ~40us; per-queue rate scales with descriptor size
            # (bytes-per-partition of the piece), and a queue round-robins
            # batches across ALL its queued transfers, so a critical
            # piece finishes promptly only if nothing is queued behind it.
            # Each need-ordered piece therefore rides its own queue
            # (scalar/gpsimd for weights, sync/vector for activations).
            nc.scalar.dma_start(wqk_sb[:, 0, 0:2, :], wq_ap[:, 0:2, :])
            nc.scalar.dma_start(wqk_sb[:, 0, 2:8, :], wq_ap[:, 2:8, :])
            nc.scalar.dma_start(wqk_sb[:, 0, 8:, :], wq_ap[:, 8:, :])
            cs0 = csp.tile([HD, 2, PROJ], BF16, tag="cs")
            nc.gpsimd.dma_start(cs0[:], cs_d.ap()[0])
            nc.scalar.dma_start(wv_sb[:], wv_ap[:])

            for ns in range(NSLICE):
                tsl = slice(ns * PROJ, (ns + 1) * PROJ)
                xt = xtp.tile([HD, NCHUNK, PROJ], BF16, tag="xt")
                if ns == 0:
                    cs_sl = cs0
                    nc.sync.dma_start(xt[:, 0:2, :], xt_ap[ns, :, 0:2, :])
                    nc.sync.dma_start(xt[:, 2:8, :], xt_ap[ns, :, 2:8, :])
                    nc.sync.dma_start(xt[:, 8:, :], xt_ap[ns, :, 8:, :])
                    # wk follows the critical x pieces on sync (its 16KB
                    # descriptors would otherwise starve them), split so
                    # the k accumulation can start on the first half
                    nc.sync.dma_start(wqk_sb[:, 1, 0:8, :], wk_ap[:, 0:8, :])
                    nc.sync.dma_start(wqk_sb[:, 1, 8:, :], wk_ap[:, 8:, :])
                else:
                    cs_sl = csp.tile([HD, 2, PROJ], BF16, tag="cs")
                    nc.sync.dma_start(xt[:], xt_ap[ns])
                    nc.scalar.dma_start(cs_sl[:], cs_d.ap()[ns])
                if ns == 2:
                    # tiny attention constants go late: their 256B
                    # descriptors would eat DMA-engine slots during the
                    # startup-critical window
                    nc.gpsimd.dma_start(ones_sb[:], ones_d[:])
                    nc.gpsimd.dma_start(mk_sb[:], mk_d[:])
                if ns == 1:
                    # wo rides the sync queue once the startup-critical
                    # pieces are through (needed only by the first
                    # output projection, ~170us in)
                    nc.sync.dma_start(wo_sb[:], wo_ap[:])

                def rope(ps, wsel, h):
                    # RoPE: dst = raw*C + swap(raw)*S.  The swap is
                    # materialized by two ScalarE half-copies; VectorE
                    # does two multiplies and one add per tile.
                    dst = qk_sb[:, wsel * HPC + h, tsl]
                    sw = ropep.tile([HD, PROJ], F32, tag="sw")
                    nc.scalar.copy(sw[0:64, :], ps[64:128, :])
                    nc.scalar.copy(sw[64:128, :], ps[0:64, :])
                    tb = ropep.tile([HD, PROJ], F32, tag="tb")
                    nc.vector.tensor_mul(dst, ps[:], cs_sl[:, 0, :])
                    nc.vector.tensor_mul(tb[:], sw[:], cs_sl[:, 1, :])
                    nc.vector.tensor_add(dst, dst, tb[:])

                if ns == 0:
                    # startup: q accumulations run in chunk phases that
                    # match the wq/xt DMA pieces, all heads per phase
                    qtiles = [pps.tile([HD, PROJ], F32, tag="ps",
                                       name=f"qps{h}")
                              for h in range(HPC)]
                    wrm = pps.tile([HD, PROJ], F32, tag="ps", name="wrm")
                    for a, b in ((0, 2), (2, 8), (8, NCHUNK)):
                        for h in range(HPC):
                            hs = slice(h * HD, (h + 1) * HD)
                            for k in range(a, b):
                                nc.tensor.matmul(
                                    qtiles[h][:], wqk_sb[:, 0, k, hs],
                                    xt[:, k, :],
                                    start=(k == 0), stop=(k == NCHUNK - 1),
                                )
                            if b == NCHUNK:
                                rope(qtiles[h], 0, h)
                        if b == 2:
                            # keep the PE busy (and the HAM clock warm)
                            # while the next wq/xt pieces land
                            for _ in range(52):
                                nc.tensor.matmul(wrm[:, 0:HD], wsrc[:],
                                                 wsrc[:], start=True,
                                                 stop=True)
                    ktiles = [pps.tile([HD, PROJ], F32, tag="ps",
                                       name=f"kps{h}")
                              for h in range(HPC)]
                    for a, b in ((0, 8), (8, NCHUNK)):
                        for h in range(HPC):
                            hs = slice(h * HD, (h + 1) * HD)
                            for k in range(a, b):
                                nc.tensor.matmul(
                                    ktiles[h][:], wqk_sb[:, 1, k, hs],
                                    xt[:, k, :],
                                    start=(k == 0), stop=(k == NCHUNK - 1),
                                )
                            if b == NCHUNK:
                                rope(ktiles[h], 1, h)
                else:
                    for wsel in range(2):
                        for h in range(HPC):
                            ps = pps.tile([HD, PROJ], F32, tag="ps")
                            hs = slice(h * HD, (h + 1) * HD)
                            for k in range(NCHUNK):
                                nc.tensor.matmul(
                                    ps[:],
                                    wqk_sb[:, wsel, k, hs],
                                    xt[:, k, :],
                                    start=(k == 0),
                                    stop=(k == NCHUNK - 1),
                                )
                            rope(ps, wsel, h)

                # ---- v projection for this slice (x^T stationary) ----
                for tc2 in range(PROJ // HD):
                    vp_t = vps.tile([HD, HPC * HD], F32, tag="vps")
                    for k in range(NCHUNK):
                        nc.tensor.matmul(
                            vp_t[:],
                            xt[:, k, tc2 * HD:(tc2 + 1) * HD],
                            wv_sb[:, k, :],
                            start=(k == 0),
                            stop=(k == NCHUNK - 1),
                        )
                    nc.scalar.copy(v_all[:, ns * (PROJ // HD) + tc2, :], vp_t[:])

        # ---- attention + interleaved output projection ----
        with ExitStack() as ph:
            ptp = ph.enter_context(tc.tile_pool(name="ptp", side="right", bufs=3))
            lrp = ph.enter_context(tc.tile_pool(name="lrp", side="right", bufs=2))
            lqp = ph.enter_context(tc.tile_pool(name="lqp", side="right", bufs=6))
            evp = ph.enter_context(tc.tile_pool(name="evp", side="right", bufs=3))
            sps = ph.enter_context(tc.tile_pool(name="sps", bufs=2, space="PSUM"))
            olps = ph.enter_context(tc.tile_pool(name="olps", bufs=2, space="PSUM"))
            wops = ph.enter_context(tc.tile_pool(name="wops", bufs=2, space="PSUM"))

            def finalize(st):
                # DVE fast-recip of the (already partition-broadcast) row
                # sums, then one DVE multiply into ctx
                lr = lrp.tile([HD, PAIR], F32, tag="lr")
                nc.vector.reciprocal_approx_fast(lr[:], st["l"])
                nc.vector.tensor_mul(
                    ctx_sb[:, st["h"], st["qsl"]], st["o"], lr[:]
                )

            def emit_outproj(j):
                # output projection for query group j (all heads final);
                # PSUM evacuations alternate ScalarE/VectorE, output DMA
                # is one 512KB transfer per 128-query tile, queues
                # alternating sync/gpsimd.
                for tck in range(PAIR // HD):
                    tsl = slice(j * PAIR + tck * HD, j * PAIR + (tck + 1) * HD)
                    ev = evp.tile([HD, D], BF16, tag="ev")
                    for es in range(D // 512):
                        esl = slice(es * 512, (es + 1) * 512)
                        ps = wops.tile([HD, 512], F32, tag="wo")
                        for h in range(HPC):
                            nc.tensor.matmul(
                                ps[:],
                                ctx_sb[:, h, tsl],
                                wo_sb[:, h, esl],
                                start=(h == 0),
                                stop=(h == HPC - 1),
                            )
                        if es % 2 == 0:
                            nc.scalar.copy(ev[:, esl], ps[:])
                        else:
                            nc.vector.tensor_copy(ev[:, esl], ps[:])
                    q = nc.sync if (2 * j + tck) % 2 == 0 else nc.gpsimd
                    q.dma_start(out_d[tsl, :], ev[:])

            def emit_ol(qinfo, st):
                # deferred p@v + l matmuls for an exp'd quad.  o and l
                # share one PSUM bank as ONE interleaved accumulation
                # group: the first pv matmul opens it (bank-wide
                # has_written clear), every later pv/l matmul
                # accumulates element-wise in its own column range.
                quad, lq = qinfo
                h = st["h"]

                def flags():
                    i = st["mm_i"]
                    st["mm_i"] += 1
                    return i == 0, i == st["nmm"] - 1

                for s, b in quad:
                    c = b["c"]
                    vsl = slice(h * HD, (h + 1) * HD)
                    sa, so = flags()
                    if b["kind"] == "full":
                        nc.tensor.matmul(
                            st["o"], v_all[:, c, vsl], st["pt"][:, s, :],
                            start=sa, stop=so, skip_group_check=True,
                        )
                    else:
                        nc.tensor.matmul(
                            st["o"][:, HD:PAIR], v_all[:, c, vsl],
                            st["pt"][:, s, 0:HD],
                            start=sa, stop=so, skip_group_check=True,
                        )
                fulls = [s for s, b in quad if b["kind"] == "full"]
                if lq is not None:
                    sa, so = flags()
                    nc.tensor.matmul(
                        st["l"], ones_sb[:], lq[:],
                        start=sa, stop=so, skip_group_check=True,
                    )
                elif len(fulls) == 1:
                    sa, so = flags()
                    nc.tensor.matmul(
                        st["l"], ones_sb[:], st["pt"][:, fulls[0], :],
                        start=sa, stop=so, skip_group_check=True,
                    )
                for s, b in quad:
                    # narrow blocks are folded into lq on the DVE; the
                    # PE fallback only fires for all-narrow quads
                    if b["kind"] == "narrow" and lq is None:
                        sa, so = flags()
                        nc.tensor.matmul(
                            st["l"][:, HD:PAIR], ones_sb[:],
                            st["pt"][:, s, 0:HD],
                            start=sa, stop=so, skip_group_check=True,
                        )
                return st["mm_i"] == st["nmm"]

            # pv/l matmuls lag up to TWO quads behind the score/exp
            # stream: at group boundaries the deferred work pads the PE
            # while the last quad's exp->fold chain completes.
            pending = []          # FIFO of ((quad, lq), st)
            fin = [None]

            def maybe_finalize(cur_st):
                fs = fin[0]
                if fs is not None and fs is not cur_st:
                    finalize(fs)
                    fin[0] = None
                    if fs["h"] == HPC - 1:
                        emit_outproj(fs["j"])

            def flush_one(cur_st):
                qinfo, pst = pending.pop(0)
                if emit_ol(qinfo, pst):
                    fin[0] = pst
                maybe_finalize(cur_st)

            prev_st = None
            # big groups first; the tiny groups (j=0,1) are slotted
            # mid-stream so their exp->fold->l latency chains hide
            # behind neighbouring groups' matmul streams
            for j in (6, 5, 4, 3, 1, 0, 2, 7):
                qsl = slice(j * PAIR, (j + 1) * PAIR)
                blocks = statuses[j]
                n = len(blocks)
                quads = [list(enumerate(blocks))[i:i + 4] for i in range(0, n, 4)]
                nmm = 0
                for quad in quads:
                    nf = sum(1 for _, b in quad if b["kind"] == "full")
                    nn = len(quad) - nf
                    nmm += len(quad) + (1 if nf >= 1 else nn)
                for h in range(HPC):
                    # only the previous group may still have deferred
                    # quads once this group's PSUM tile is allocated
                    # (olps has 2 buffers)
                    while pending and pending[0][1] is not prev_st:
                        flush_one(None)
                    ol_ps = olps.tile([HD, 2 * PAIR], F32, tag="ol")
                    pt = ptp.tile([HD, NCHUNK, PAIR], BF16, tag="pt")
                    st = {"o": ol_ps[:, 0:PAIR], "l": ol_ps[:, PAIR:2 * PAIR],
                          "pt": pt, "h": h, "qsl": qsl, "j": j,
                          "mm_i": 0, "nmm": nmm}
                    for qi, quad in enumerate(quads):
                        s_ps = sps.tile([HD, 4, PAIR], F32, tag="s")
                        for s, b in quad:
                            c = b["c"]
                            ksl = slice(c * HD, (c + 1) * HD)
                            sl = s - qi * 4
                            if b["kind"] == "full":
                                nc.tensor.matmul(
                                    s_ps[:, sl, :],
                                    qk_sb[:, HPC + h, ksl],
                                    qk_sb[:, h, qsl],
                                    start=True, stop=True,
                                )
                            else:
                                nc.tensor.matmul(
                                    s_ps[:, sl, 0:HD],
                                    qk_sb[:, HPC + h, ksl],
                                    qk_sb[:, h, j * PAIR + HD:(j + 1) * PAIR],
                                    start=True, stop=True,
                                )
                        # exp: ONE ACT op per quad, full-width.  A narrow
                        # slot's upper 128 columns hold PSUM garbage whose
                        # exp lands in pt columns no consumer ever reads
                        # (pv/l/mask touch only its 0:128 range).
                        nc.scalar.activation(
                            pt[:, qi * 4:qi * 4 + len(quad), :],
                            s_ps[:, 0:len(quad), :], EXP,
                        )
                        # multiplicative exp-mask per query half
                        # (exp(s+m) == exp(s)*exp(m)), off the exp chain
                        for s, b in quad:
                            if b["kind"] == "full":
                                if b["lo"] >= 0:
                                    nc.vector.tensor_mul(
                                        pt[:, s, 0:HD], pt[:, s, 0:HD],
                                        mk_sb[:, b["lo"], :])
                                if b["hi"] >= 0:
                                    nc.vector.tensor_mul(
                                        pt[:, s, HD:PAIR], pt[:, s, HD:PAIR],
                                        mk_sb[:, b["hi"], :])
                            else:
                                if b["hi"] >= 0:
                                    nc.vector.tensor_mul(
                                        pt[:, s, 0:HD], pt[:, s, 0:HD],
                                        mk_sb[:, b["hi"], :])
                        # DVE quad-fold of full blocks for the l matmul;
                        # narrow blocks fold their 128 columns into the
                        # upper half of lq (sparing the PE an l matmul)
                        lq = None
                        nf = len([1 for _, b in quad if b["kind"] == "full"])
                        narrows = [s for s, b in quad if b["kind"] == "narrow"]
                        s0 = quad[0][0]
                        if nf == 4:
                            t2 = lqp.tile([HD, 2, PAIR], BF16, tag="t2")
                            nc.vector.tensor_add(
                                t2[:], pt[:, s0:s0 + 2, :], pt[:, s0 + 2:s0 + 4, :])
                            lq = lqp.tile([HD, PAIR], BF16, tag="lq")
                            nc.vector.tensor_add(lq[:], t2[:, 0, :], t2[:, 1, :])
                        elif nf == 3:
                            lq = lqp.tile([HD, PAIR], BF16, tag="lq")
                            nc.vector.tensor_add(
                                lq[:], pt[:, s0, :], pt[:, s0 + 1, :])
                            nc.vector.tensor_add(lq[:], lq[:], pt[:, s0 + 2, :])
                        elif nf == 2:
                            lq = lqp.tile([HD, PAIR], BF16, tag="lq")
                            nc.vector.tensor_add(
                                lq[:], pt[:, s0, :], pt[:, s0 + 1, :])
                        elif nf == 1 and narrows:
                            lq = lqp.tile([HD, PAIR], BF16, tag="lq")
                            nc.vector.tensor_copy(lq[:], pt[:, s0, :])
                        if lq is not None:
                            for s in narrows:
                                nc.vector.tensor_add(
                                    lq[:, HD:PAIR], lq[:, HD:PAIR],
                                    pt[:, s, 0:HD])
                        # queue this quad's pv/l; flush the oldest once
                        # more than two are deferred
                        pending.append(((quad, lq), st))
                        while len(pending) > 3:
                            flush_one(st)
                        maybe_finalize(st)
                    prev_st = st
            while pending:
                flush_one(None)
            maybe_finalize(None)
    nc.compile()
    return nc


_PERM = np.concatenate(
    [np.concatenate([np.arange(0, HD, 2), np.arange(1, HD, 2)]) + h * HD
     for h in range(HPC)]
)


def prepare(x, freqs, mask, wq, wk, wv, wo):
    """Host-side sharding/prep. Returns (nc, in_maps)."""
    x = np.asarray(x, np.float32)
    freqs = np.asarray(freqs, np.float32)
    mask = np.asarray(mask, np.float32)
    wq, wk, wv, wo = (np.asarray(w, np.float32) for w in (wq, wk, wv, wo))

    statuses, maskt = _mask_structure(mask)
    nc = _build_program(statuses, maskt.shape[1])

    scale = np.float32(1.0 / np.sqrt(HD))
    cos = np.ascontiguousarray(freqs[:, :, 0].T)  # (64, T)
    sin = np.ascontiguousarray(freqs[:, :, 1].T)
    cs = np.empty((HD, 2, T), np.float32)
    cs[0:64, 0, :] = cos
    cs[64:128, 0, :] = cos
    cs[0:64, 1, :] = -sin
    cs[64:128, 1, :] = sin
    # per-slice layout so each slice's DMA is one contiguous
    # 2KB-per-partition run
    cs = np.ascontiguousarray(
        cs.reshape(HD, 2, NSLICE, PROJ).transpose(2, 0, 1, 3)
    ).astype(NPBF16)

    ones_sq = np.ones((HD, HD), NPBF16)
    maskt_bf = maskt.astype(NPBF16)

    def pshuf_w(w):
        # [D, E] -> [HD, NCHUNK, E]: partition-major so each SBUF
        # partition's content is one contiguous DRAM run
        return np.ascontiguousarray(
            w.reshape(NCHUNK, HD, w.shape[1]).transpose(1, 0, 2)
        ).astype(NPBF16)

    def pshuf_x(xb):
        # x[b].T [D, T] -> [NSLICE, HD, NCHUNK, PROJ]
        xT = xb.T.reshape(NCHUNK, HD, NSLICE, PROJ)
        return np.ascontiguousarray(xT.transpose(2, 1, 0, 3)).astype(NPBF16)

    xts = [pshuf_x(x[b]) for b in range(B)]

    in_maps = []
    for core in range(8):
        b, g = core // 4, core % 4
        cols = slice(g * HPC * HD, (g + 1) * HPC * HD)
        wot = wo.T[cols, :].reshape(HPC, HD, D).transpose(1, 0, 2)
        in_maps.append({
            "xts": xts[b],
            "wqt": pshuf_w((wq.T[:, cols] * scale)[:, _PERM]),
            "wkt": pshuf_w(wk.T[:, cols][:, _PERM]),
            "wvt": pshuf_w(wv.T[:, cols]),
            "wot": np.ascontiguousarray(wot).astype(NPBF16),
            "cs": cs,
            "maskt": maskt_bf,
            "ones_sq": ones_sq,
        })
    return nc, in_maps


def run(x, freqs, mask, wq, wk, wv, wo, **spmd_kwargs):
    nc, in_maps = prepare(x, freqs, mask, wq, wk, wv, wo)
    res = run_bass_kernel_spmd(nc, in_maps, list(range(8)), **spmd_kwargs)
    parts = [np.asarray(res.results[c]["out"], np.float32) for c in range(8)]
    out = np.stack([
        parts[b * 4] + parts[b * 4 + 1] + parts[b * 4 + 2] + parts[b * 4 + 3]
        for b in range(B)
    ])
    return out, res


def kernel(x, freqs, mask, wq, wk, wv, wo):
    out, _ = run(x, freqs, mask, wq, wk, wv, wo)
    return out
